# revision 6
# baseline (speedup 1.0000x reference)
"""Trainium2 Bass kernel: DiT block with cross-attention (nn_DiTBlock_CrossAttn).

Sharding: pure data-parallel over batch. B=8 batch elements -> 8 NeuronCores,
one batch element per core, no collectives. Each core runs the full block:
adaLN -> self-attn -> cross-attn -> FFN (exact GELU).

Layout: activations kept feature-major ("transposed", [feature_part, chunk, token])
so every projection is matmul(lhsT=W[din,dout], rhs=actT[din,n]) with weights in
their natural DRAM layout. Attention uses the S^T orientation with a fused
ones-column in V for the softmax denominator (softmax without max subtraction is
safe: |logits| < ~5 for this problem family). Matmuls run in bf16 (weights are
pre-cast on host), accumulation and residual stream stay fp32.
"""
import contextlib

import numpy as np
import ml_dtypes

import concourse.bass as bass
import concourse.tile as tile
import concourse.mybir as mybir
from concourse import bacc
from concourse.bass_utils import run_bass_kernel_spmd
from concourse.masks import make_identity

P = 128
N = 1024            # tokens
D = 1024            # hidden
KD = D // P         # 8 feature chunks of hidden
NT = N // P         # 8 token tiles
H = 16              # heads
HD = 64             # head dim
S = 256             # context tokens
ST = S // P         # 2
CD = 512            # context dim
CKD = CD // P       # 4
MLP = 4096
MT = MLP // P       # 32
EPS = 1e-6
ASCALE = 0.125      # 1/sqrt(HD)
NCORES = 8

F32 = mybir.dt.float32
BF16 = mybir.dt.bfloat16
AF = mybir.ActivationFunctionType
OP = mybir.AluOpType


def _wcols(w):
    """[din, dout] dram AP -> [p, ko, dout] (feature-chunked lhsT view)."""
    return w.rearrange("(ko p) f -> p ko f", p=P)


def build_nc(taps=()):
    nc = bacc.Bacc("TRN2", target_bir_lowering=False, debug=False)

    d = {}
    d['x'] = nc.dram_tensor("x", [N, D], F32, kind="ExternalInput").ap()
    d['c'] = nc.dram_tensor("c", [D], F32, kind="ExternalInput").ap()
    d['context'] = nc.dram_tensor("context", [S, CD], F32, kind="ExternalInput").ap()
    for nm, sh in [("w_qkv", [D, 3 * D]), ("w_so", [D, D]), ("w_cq", [D, D]),
                   ("w_ck", [CD, D]), ("w_cv", [CD, D]), ("w_co", [D, D]),
                   ("w1", [D, MLP]), ("w2", [MLP, D]), ("w_ada", [D, 6 * D])]:
        d[nm] = nc.dram_tensor(nm, sh, BF16, kind="ExternalInput").ap()
    for nm, sh in [("b_qkv", [3 * D]), ("b_so", [D]), ("b_cq", [D]), ("b_ck", [D]),
                   ("b_cv", [D]), ("b_co", [D]), ("b1", [MLP]), ("b2", [D]),
                   ("b_ada", [6 * D])]:
        d[nm] = nc.dram_tensor(nm, sh, F32, kind="ExternalInput").ap()
    out = nc.dram_tensor("out_x", [N, D], F32, kind="ExternalOutput").ap()
    srows = nc.dram_tensor("srows", [40, N], F32).ap()
    g_dram = nc.dram_tensor("g_dram", [MT, P, N], BF16).ap()

    tap_shapes = {
        "ada": ([P, 48], F32), "h1": ([P, KD, N], BF16),
        "q": ([P, KD, N], BF16), "k": ([P, KD, N], BF16),
        "v65": ([P, NT, H, 65], BF16), "saO": ([P, KD, N], BF16),
        "x2": ([P, KD, N], F32), "h2": ([P, KD, N], BF16),
        "cq": ([P, KD, N], BF16), "ck": ([P, KD, S], BF16),
        "cv65": ([P, ST, H, 65], BF16), "caO": ([P, KD, N], BF16),
        "x3": ([P, KD, N], F32), "h3": ([P, KD, N], BF16),
        "xT": ([P, KD, N], F32),
    }
    tap_aps = {nm: nc.dram_tensor(f"dbg_{nm}", *tap_shapes[nm], kind="ExternalOutput").ap()
               for nm in taps}

    with tile.TileContext(nc) as tc:
        _emit(nc, tc, d, out, srows, g_dram, tap_aps)
    nc.compile()
    return nc


def _emit(nc, tc, d, out, srows, g_dram, tap_aps={}):
    def tap(nm, t):
        if nm in tap_aps:
            nc.sync.dma_start(tap_aps[nm], t[:])

    gl = contextlib.ExitStack()          # global pools, whole-kernel lifetime
    with gl:
        const = gl.enter_context(tc.tile_pool(name="const", bufs=1))
        resid = gl.enter_context(tc.tile_pool(name="resid", bufs=2))
        wpool = gl.enter_context(tc.tile_pool(name="wpool", bufs=2))
        bigbf = gl.enter_context(tc.tile_pool(name="bigbf", bufs=3))

        # ---------- constants ----------
        ident = const.tile([P, P], F32, tag="ident")
        make_identity(nc, ident)
        ones_bf = const.tile([P, 1], BF16, tag="ones_bf")
        nc.vector.memset(ones_bf[:], 1.0)
        eps_t = const.tile([P, 1], F32, tag="eps")
        nc.vector.memset(eps_t[:], EPS)

        ctxT = const.tile([P, CKD, S], BF16, tag="ctxT")
        ada = const.tile([P, 48], F32, tag="ada")
        splus = const.tile([P, 24], F32, tag="splus")
        xT = resid.tile([P, KD, N], F32, tag="resid")

        # ---------- staging scope ----------
        st = contextlib.ExitStack()
        stg = st.enter_context(tc.tile_pool(name="stg", bufs=2))
        ps_t = st.enter_context(tc.tile_pool(name="ps_t", bufs=2, space="PSUM"))

        def bias_T(name, brow, width):
            stage = stg.tile([width, P], F32, tag="btmp")
            nc.gpsimd.dma_start(stage[:], brow.rearrange("(a p) -> a p", p=P))
            ps = ps_t.tile([P, 512], F32, tag="pst")
            nc.tensor.transpose(ps[:, 0:width], stage[:], ident[0:width, 0:width])
            t = const.tile([P, width], F32, tag=f"bT_{name}")
            nc.vector.tensor_copy(t[:], ps[:, 0:width])
            return t

        b_qkvT = bias_T("qkv", d['b_qkv'], 24)
        b_soT = bias_T("so", d['b_so'], KD)
        b_cqT = bias_T("cq", d['b_cq'], KD)
        b_ckT = bias_T("ck", d['b_ck'], KD)
        b_coT = bias_T("co", d['b_co'], KD)
        b1T = bias_T("b1", d['b1'], MT)
        b_adaT = bias_T("ada", d['b_ada'], 48)
        b2T = bias_T("b2", d['b2'], KD)

        # x -> xT (feature-major, fp32, via PE transpose)
        for i in range(NT):
            xs = stg.tile([P, D], F32, tag="xstage")
            nc.sync.dma_start(xs[:], d['x'][i * P:(i + 1) * P, :])
            for jg in range(2):
                ps = ps_t.tile([P, 512], F32, tag="pst")
                for j4 in range(4):
                    j = jg * 4 + j4
                    nc.tensor.transpose(ps[:, j4 * 128:(j4 + 1) * 128],
                                        xs[:, j * 128:(j + 1) * 128], ident[:])
                nc.vector.tensor_copy(
                    xT[:, jg * 4:(jg + 1) * 4, i * P:(i + 1) * P],
                    ps.rearrange("p (a b) -> p a b", a=4))

        # context -> ctxT (bf16)
        for i in range(ST):
            cs = stg.tile([P, D], F32, tag="xstage")
            nc.sync.dma_start(cs[:, 0:CD], d['context'][i * P:(i + 1) * P, :])
            ps = ps_t.tile([P, 512], F32, tag="pst")
            for j in range(4):
                nc.tensor.transpose(ps[:, j * 128:(j + 1) * 128],
                                    cs[:, j * 128:(j + 1) * 128], ident[:])
            nc.vector.tensor_copy(
                ctxT[:, :, i * P:(i + 1) * P],
                ps.rearrange("p (a b) -> p a b", a=4))

        # c -> silu(c)^T (bf16, feature-major [P, KD])
        cst = stg.tile([KD, P], F32, tag="cstage")
        nc.gpsimd.dma_start(cst[:], d['c'].rearrange("(a p) -> a p", p=P))
        csil = stg.tile([KD, P], F32, tag="cstage")
        nc.scalar.activation(csil[:], cst[:], AF.Silu)
        pcs = ps_t.tile([P, 512], F32, tag="pst")
        nc.tensor.transpose(pcs[:, 0:KD], csil[:], ident[0:KD, 0:KD])
        silu_cT = const.tile([P, KD], BF16, tag="silu_cT")
        nc.vector.tensor_copy(silu_cT[:], pcs[:, 0:KD])

        # ada = silu(c) @ w_ada + b_ada  -> feature-major [P, 48]
        wada = _wcols(d['w_ada'])
        for blk in range(12):
            wb = wpool.tile([P, KD, 512], BF16, tag="wblk")
            nc.sync.dma_start(wb[:], wada[:, :, blk * 512:(blk + 1) * 512])
            for t4 in range(4):
                t = blk * 4 + t4
                ps = ps_t.tile([P, 512], F32, tag="pst")
                for k in range(KD):
                    nc.tensor.matmul(ps[:, 0:1], wb[:, k, t4 * 128:(t4 + 1) * 128],
                                     silu_cT[:, k:k + 1],
                                     start=(k == 0), stop=(k == KD - 1))
                nc.vector.tensor_copy(ada[:, t:t + 1], ps[:, 0:1])
        nc.vector.tensor_add(ada[:], ada[:], b_adaT[:])
        for g in range(3):
            nc.vector.tensor_scalar_add(splus[:, g * 8:(g + 1) * 8],
                                        ada[:, g * 16 + 8:g * 16 + 16], 1.0)
        tap("ada", ada)
        tap("xT", xT)
        st.close()

        # ---------- LN + modulate (self-contained pool scope) ----------
        def ln_mod(x_in, g):
            h_out = bigbf.tile([P, KD, N], BF16, tag="big")
            ls = contextlib.ExitStack()
            with ls:
                lnb = ls.enter_context(tc.tile_pool(name="lnb", bufs=2))
                lrows = ls.enter_context(tc.tile_pool(name="lrows", bufs=3))
                lbc = ls.enter_context(tc.tile_pool(name="lbc", bufs=2))
                ps_ln = ls.enter_context(tc.tile_pool(name="ps_ln", bufs=2, space="PSUM"))
                mps = ps_ln.tile([1, N], F32, tag="lnp")
                sps = ps_ln.tile([1, N], F32, tag="lnp")
                for k in range(KD):
                    xbf = lnb.tile([P, N], BF16, tag="lnxbf")
                    nc.vector.tensor_copy(xbf[:], x_in[:, k])
                    sq = lnb.tile([P, N], BF16, tag="lnsq")
                    nc.vector.tensor_mul(sq[:], xbf[:], xbf[:])
                    for half in range(2):
                        hs = slice(half * 512, (half + 1) * 512)
                        nc.tensor.matmul(mps[:, hs], ones_bf[:], xbf[:, hs],
                                         start=(k == 0), stop=(k == KD - 1))
                        nc.tensor.matmul(sps[:, hs], ones_bf[:], sq[:, hs],
                                         start=(k == 0), stop=(k == KD - 1))
                mu = lrows.tile([1, N], F32, tag="row")
                nc.vector.tensor_scalar_mul(mu[:], mps[:], 1.0 / D)
                var = lrows.tile([1, N], F32, tag="row")
                nc.vector.tensor_scalar_mul(var[:], sps[:], 1.0 / D)
                nc.vector.tensor_mul(mps[:, 0:N], mu[:], mu[:])
                nc.vector.tensor_sub(var[:], var[:], mps[:, 0:N])
                sd = lrows.tile([1, N], F32, tag="row")
                nc.scalar.activation(sd[:], var[:], AF.Sqrt, bias=eps_t[0:1])
                nc.vector.reciprocal(sd[:], sd[:])
                nc.gpsimd.dma_start(srows[2 * g:2 * g + 1, :], mu[:])
                nc.gpsimd.dma_start(srows[2 * g + 1:2 * g + 2, :], sd[:])
                mu_b = lbc.tile([P, N], F32, tag="bcast")
                nc.gpsimd.dma_start(mu_b[:], srows[2 * g:2 * g + 1, :].partition_broadcast(P))
                rstd_b = lbc.tile([P, N], F32, tag="bcast")
                nc.gpsimd.dma_start(rstd_b[:], srows[2 * g + 1:2 * g + 2, :].partition_broadcast(P))
                for k in range(KD):
                    t1 = lnb.tile([P, N], F32, tag="lnt1")
                    nc.vector.tensor_sub(t1[:], x_in[:, k], mu_b[:])
                    nc.vector.tensor_mul(t1[:], t1[:], rstd_b[:])
                    nc.vector.tensor_scalar(h_out[:, k], t1[:],
                                            splus[:, g * 8 + k:g * 8 + k + 1],
                                            ada[:, g * 16 + k:g * 16 + k + 1],
                                            OP.mult, OP.add)
            return h_out

        # ---------- generic transposed projection (512-wide weight blocks) ----
        def proj_T(ps_mm, w_cols, kdin, act_bf, n_free, dout, evict):
            nhalf = max(1, n_free // 512)
            for blk in range(dout // 512):
                wb = wpool.tile([P, kdin, 512], BF16, tag="wblk")
                nc.sync.dma_start(wb[:], w_cols[:, :, blk * 512:(blk + 1) * 512])
                for t4 in range(4):
                    ps = ps_mm.tile([P, N], F32, tag="pmm")
                    for half in range(nhalf):
                        hs = slice(half * 512, half * 512 + min(512, n_free))
                        for k in range(kdin):
                            nc.tensor.matmul(ps[:, hs],
                                             wb[:, k, t4 * 128:(t4 + 1) * 128],
                                             act_bf[:, k, hs],
                                             start=(k == 0), stop=(k == kdin - 1))
                    evict(blk * 4 + t4, ps)

        # ---------- token-major V projection (fused ones column) ----------
        def proj_V(ps_mm, w_cols, kdin, act_bf, m_tiles, v65, bias_b):
            for blk in range(2):
                wb = wpool.tile([P, kdin, 512], BF16, tag="wblk")
                nc.sync.dma_start(wb[:], w_cols[:, :, blk * 512:(blk + 1) * 512])
                for i in range(m_tiles):
                    ps = ps_mm.tile([P, N], F32, tag="pmm")
                    for k in range(kdin):
                        nc.tensor.matmul(ps[:, 0:512],
                                         act_bf[:, k, i * 128:(i + 1) * 128],
                                         wb[:, k, :],
                                         start=(k == 0), stop=(k == kdin - 1))
                    nc.vector.tensor_add(
                        v65[:, i, blk * 8:(blk + 1) * 8, 0:64],
                        ps[:, 0:512].rearrange("p (h e) -> p h e", h=8),
                        bias_b[:, blk * 512:(blk + 1) * 512]
                        .rearrange("p (h e) -> p h e", h=8))
            nc.vector.memset(v65[:, :, :, 64:65], 1.0)

        # ---------- attention core (self-contained pool scope) ----------
        def attention(q_T, kv_T, v65, m_tiles, o_bf, srow_base):
            at = contextlib.ExitStack()
            with at:
                expp = at.enter_context(tc.tile_pool(name="expp", bufs=2))
                arows = at.enter_context(tc.tile_pool(name="arows", bufs=2))
                rb = at.enter_context(tc.tile_pool(name="rb", bufs=2))
                ps_lg = at.enter_context(tc.tile_pool(name="ps_lg", bufs=2, space="PSUM"))
                ps_pv = at.enter_context(tc.tile_pool(name="ps_pv", bufs=2, space="PSUM"))
                for h in range(H):
                    pr, off = h // 2, (h % 2) * 64
                    pv = ps_pv.tile([65, N], F32, tag="pv")
                    for mt in range(m_tiles):
                        lg = ps_lg.tile([P, N], F32, tag="lg")
                        for half in range(2):
                            hs = slice(half * 512, (half + 1) * 512)
                            nc.tensor.matmul(
                                lg[:, hs],
                                kv_T[off:off + 64, pr, mt * 128:(mt + 1) * 128],
                                q_T[off:off + 64, pr, hs],
                                start=True, stop=True)
                        ex = expp.tile([P, N], BF16, tag="expT")
                        nc.scalar.activation(ex[:], lg[:], AF.Exp, scale=ASCALE)
                        for half in range(2):
                            hs = slice(half * 512, (half + 1) * 512)
                            nc.tensor.matmul(pv[:, hs], v65[:, mt, h, :], ex[:, hs],
                                             start=(mt == 0), stop=(mt == m_tiles - 1))
                    rec = arows.tile([1, N], F32, tag="row")
                    nc.vector.reciprocal(rec[:], pv[64:65, :])
                    r = srow_base + h
                    nc.gpsimd.dma_start(srows[r:r + 1, :], rec[:])
                    rbt = rb.tile([64, N], F32, tag="rbt")
                    nc.gpsimd.dma_start(rbt[:], srows[r:r + 1, :].partition_broadcast(64))
                    nc.vector.tensor_mul(o_bf[off:off + 64, pr, :], pv[0:64, :], rbt[:])

        # ================= self-attention =================
        h1 = ln_mod(xT, 0)

        qT = bigbf.tile([P, KD, N], BF16, tag="big")
        kT = bigbf.tile([P, KD, N], BF16, tag="big")

        sa_es = contextlib.ExitStack()
        vp = sa_es.enter_context(tc.tile_pool(name="vp", bufs=1))
        vb = sa_es.enter_context(tc.tile_pool(name="vb", bufs=1))
        v65 = vp.tile([P, NT, H, 65], BF16, tag="v65")
        vbias_b = vb.tile([P, D], F32, tag="vbias")
        nc.gpsimd.dma_start(vbias_b[:],
                            d['b_qkv'][2 * D:3 * D][None, :].partition_broadcast(P))

        qkv_ps = contextlib.ExitStack()
        ps_mm = qkv_ps.enter_context(tc.tile_pool(name="ps_mm", bufs=2, space="PSUM"))

        def ev_qk(t, ps):
            dst = qT if t < 8 else kT
            nc.scalar.activation(dst[:, t % 8, :], ps[:], AF.Identity,
                                 bias=b_qkvT[:, t:t + 1])
        proj_T(ps_mm, _wcols(d['w_qkv'])[:, :, 0:2 * D], KD, h1, N, 2 * D, ev_qk)
        proj_V(ps_mm, _wcols(d['w_qkv'])[:, :, 2 * D:3 * D], KD, h1, NT, v65, vbias_b)
        tap("h1", h1); tap("q", qT); tap("k", kT); tap("v65", v65)
        qkv_ps.close()

        saO = bigbf.tile([P, KD, N], BF16, tag="big")
        attention(qT, kT, v65, NT, saO, 6)
        tap("saO", saO)
        sa_es.close()

        x2T = resid.tile([P, KD, N], F32, tag="resid")
        so_ps = contextlib.ExitStack()
        ps_mm = so_ps.enter_context(tc.tile_pool(name="ps_mm", bufs=2, space="PSUM"))

        def ev_so(t, ps):
            nc.vector.tensor_scalar_add(x2T[:, t, :], ps[:], b_soT[:, t:t + 1])
            nc.vector.tensor_add(x2T[:, t, :], x2T[:, t, :], xT[:, t, :])
        proj_T(ps_mm, _wcols(d['w_so']), KD, saO, N, D, ev_so)
        tap("x2", x2T)
        so_ps.close()

        # ================= cross-attention =================
        h2 = ln_mod(x2T, 1)

        cqT = bigbf.tile([P, KD, N], BF16, tag="big")

        ca_es = contextlib.ExitStack()
        kp = ca_es.enter_context(tc.tile_pool(name="kp", bufs=1))
        vp = ca_es.enter_context(tc.tile_pool(name="vp2", bufs=1))
        vb = ca_es.enter_context(tc.tile_pool(name="vb2", bufs=1))
        ckT = kp.tile([P, KD, S], BF16, tag="ckT")
        cv65 = vp.tile([P, ST, H, 65], BF16, tag="cv65")
        cvbias_b = vb.tile([P, D], F32, tag="cvbias")
        nc.gpsimd.dma_start(cvbias_b[:], d['b_cv'][None, :].partition_broadcast(P))

        ca_ps = contextlib.ExitStack()
        ps_mm = ca_ps.enter_context(tc.tile_pool(name="ps_mm", bufs=2, space="PSUM"))

        def ev_cq(t, ps):
            nc.scalar.activation(cqT[:, t, :], ps[:], AF.Identity,
                                 bias=b_cqT[:, t:t + 1])
        proj_T(ps_mm, _wcols(d['w_cq']), KD, h2, N, D, ev_cq)
        tap("h2", h2); tap("cq", cqT)

        def ev_ck(t, ps):
            nc.scalar.activation(ckT[:, t, :], ps[:, 0:S], AF.Identity,
                                 bias=b_ckT[:, t:t + 1])
        proj_T(ps_mm, _wcols(d['w_ck']), CKD, ctxT, S, D, ev_ck)
        proj_V(ps_mm, _wcols(d['w_cv']), CKD, ctxT, ST, cv65, cvbias_b)
        tap("ck", ckT); tap("cv65", cv65)
        ca_ps.close()

        caO = bigbf.tile([P, KD, N], BF16, tag="big")
        attention(cqT, ckT, cv65, ST, caO, 22)
        tap("caO", caO)
        ca_es.close()

        x3T = resid.tile([P, KD, N], F32, tag="resid")
        co_ps = contextlib.ExitStack()
        ps_mm = co_ps.enter_context(tc.tile_pool(name="ps_mm", bufs=2, space="PSUM"))

        def ev_co(t, ps):
            nc.vector.tensor_scalar_add(x3T[:, t, :], ps[:], b_coT[:, t:t + 1])
            nc.vector.tensor_add(x3T[:, t, :], x3T[:, t, :], x2T[:, t, :])
        proj_T(ps_mm, _wcols(d['w_co']), KD, caO, N, D, ev_co)
        tap("x3", x3T)
        co_ps.close()

        # ================= FFN =================
        h3 = ln_mod(x3T, 2)
        # fold b2 into the residual before the final transpose-accumulate
        for k in range(KD):
            nc.vector.tensor_scalar_add(x3T[:, k, :], x3T[:, k, :], b2T[:, k:k + 1])

        w1_es = contextlib.ExitStack()
        gstage = w1_es.enter_context(tc.tile_pool(name="gstage", bufs=3))
        ps_mm = w1_es.enter_context(tc.tile_pool(name="ps_mm", bufs=2, space="PSUM"))

        def ev_g(t, ps):
            gst = gstage.tile([P, N], BF16, tag="gst")
            nc.scalar.activation(gst[:], ps[:], AF.Gelu, bias=b1T[:, t:t + 1])
            nc.sync.dma_start(g_dram[t], gst[:])
        proj_T(ps_mm, _wcols(d['w1']), KD, h3, N, MLP, ev_g)
        tap("h3", h3)
        w1_es.close()

        w2_es = contextlib.ExitStack()
        ghp = w2_es.enter_context(tc.tile_pool(name="ghp", bufs=1))
        outst = w2_es.enter_context(tc.tile_pool(name="outst", bufs=2))
        ps_tt = w2_es.enter_context(tc.tile_pool(name="ps_tt", bufs=2, space="PSUM"))
        ps_po = w2_es.enter_context(tc.tile_pool(name="ps_po", bufs=4, space="PSUM"))

        # token-major copy of x3 (+b2), built via PE transposes into bigbf slots
        xtok = [bigbf.tile([P, 4, D], F32, tag="big", name=f"xtok{_i}")
                for _i in range(2)]
        for i in range(NT):
            dst = xtok[i // 4]
            for jg in range(2):
                ps = ps_tt.tile([P, 512], F32, tag="ptt")
                for j4 in range(4):
                    j = jg * 4 + j4
                    nc.tensor.transpose(ps[:, j4 * 128:(j4 + 1) * 128],
                                        x3T[:, j, i * P:(i + 1) * P], ident[:])
                nc.vector.tensor_copy(dst[:, i % 4, jg * 512:(jg + 1) * 512], ps[:])

        w2cols = d['w2'].rearrange("(ko p) f -> p ko f", p=P)
        for nh in range(2):
            gh = ghp.tile([P, MT, 512], BF16, tag="gh")
            nc.sync.dma_start(gh[:], g_dram[:, :, nh * 512:(nh + 1) * 512]
                              .rearrange("m p n -> p m n"))
            for dq in range(4):
                w2q = wpool.tile([P, MT, 256], BF16, tag="wblk")
                nc.sync.dma_start(w2q[:], w2cols[:, :, dq * 256:(dq + 1) * 256])
                for i4 in range(4):
                    i = nh * 4 + i4
                    po = ps_po.tile([P, 256], F32, tag="po")
                    for k in range(MT):
                        nc.tensor.matmul(po[:], gh[:, k, i4 * 128:(i4 + 1) * 128],
                                         w2q[:, k, :],
                                         start=(k == 0), stop=(k == MT - 1))
                    ost = outst.tile([P, 256], F32, tag="ost")
                    nc.vector.tensor_add(
                        ost[:], po[:],
                        xtok[i // 4][:, i % 4, dq * 256:(dq + 1) * 256])
                    nc.sync.dma_start(out[i * P:(i + 1) * P, dq * 256:(dq + 1) * 256],
                                      ost[:])
        w2_es.close()


_NC = None


def _get_nc():
    global _NC
    if _NC is None:
        _NC = build_nc()
    return _NC


def make_in_maps(inputs):
    wnames = ["w_qkv", "w_so", "w_cq", "w_ck", "w_cv", "w_co", "w1", "w2", "w_ada"]
    bnames = ["b_qkv", "b_so", "b_cq", "b_ck", "b_cv", "b_co", "b1", "b2", "b_ada"]
    shared = {}
    for nm in wnames:
        shared[nm] = np.ascontiguousarray(
            np.asarray(inputs[nm]).astype(ml_dtypes.bfloat16))
    for nm in bnames:
        shared[nm] = np.ascontiguousarray(np.asarray(inputs[nm], dtype=np.float32))
    x = np.asarray(inputs['x'], dtype=np.float32)
    c = np.asarray(inputs['c'], dtype=np.float32)
    ctxt = np.asarray(inputs['context'], dtype=np.float32)
    in_maps = []
    for i in range(NCORES):
        m = dict(shared)
        m['x'] = np.ascontiguousarray(x[i])
        m['c'] = np.ascontiguousarray(c[i])
        m['context'] = np.ascontiguousarray(ctxt[i])
        in_maps.append(m)
    return in_maps


def kernel(**inputs):
    nc = _get_nc()
    in_maps = make_in_maps(inputs)
    res = run_bass_kernel_spmd(nc, in_maps, core_ids=list(range(NCORES)))
    return np.stack([res.results[i]["out_x"] for i in range(NCORES)]).astype(np.float32)


if __name__ == "__main__":
    data = np.load("/root/problem/inputs.npz")
    out = kernel(**{k: data[k] for k in data.files})
    gold = np.load("/root/problem/gold64.npy")
    err = np.abs(out - gold)
    print("max abs err:", err.max(), " rel:", err.max() / np.abs(gold).max())


# revision 10
# speedup vs baseline: 3.0725x; 3.0725x over previous
"""Trainium2 Bass kernel: DiT block with cross-attention (nn_DiTBlock_CrossAttn).

Sharding: pure data-parallel over batch. B=8 batch elements -> 8 NeuronCores,
one batch element per core, no collectives. Each core runs the full block:
adaLN -> self-attn -> cross-attn -> FFN (exact GELU).

Layout: activations kept feature-major ("transposed", [feature_part, chunk, token])
so every projection is matmul(lhsT=W[din,dout], rhs=actT[din,n]) with weights in
their natural DRAM layout. Attention uses the S^T orientation with a fused
ones-column in V for the softmax denominator (softmax without max subtraction is
safe: |logits| < ~5 for this problem family). Matmuls run in bf16 (weights are
pre-cast on host), accumulation and residual stream stay fp32.
"""
import contextlib

import numpy as np
import ml_dtypes

import concourse.bass as bass
import concourse.tile as tile
import concourse.mybir as mybir
from concourse import bacc
from concourse.bass_utils import run_bass_kernel_spmd
from concourse.masks import make_identity

P = 128
N = 1024            # tokens
D = 1024            # hidden
KD = D // P         # 8 feature chunks of hidden
NT = N // P         # 8 token tiles
H = 16              # heads
HD = 64             # head dim
S = 256             # context tokens
ST = S // P         # 2
CD = 512            # context dim
CKD = CD // P       # 4
MLP = 4096
MT = MLP // P       # 32
EPS = 1e-6
ASCALE = 0.125      # 1/sqrt(HD)
NCORES = 8

F32 = mybir.dt.float32
BF16 = mybir.dt.bfloat16
AF = mybir.ActivationFunctionType
OP = mybir.AluOpType


def _wcols(w):
    """[din, dout] dram AP -> [p, ko, dout] (feature-chunked lhsT view)."""
    return w.rearrange("(ko p) f -> p ko f", p=P)


def build_nc(taps=(), upto='full'):
    nc = bacc.Bacc("TRN2", target_bir_lowering=False, debug=False)

    d = {}
    d['x'] = nc.dram_tensor("x", [N, D], F32, kind="ExternalInput").ap()
    d['c'] = nc.dram_tensor("c", [D], F32, kind="ExternalInput").ap()
    d['context'] = nc.dram_tensor("context", [S, CD], F32, kind="ExternalInput").ap()
    for nm, sh in [("w_qkv", [D, 3 * D]), ("w_so", [D, D]), ("w_cq", [D, D]),
                   ("w_ck", [CD, D]), ("w_cv", [CD, D]), ("w_co", [D, D]),
                   ("w1", [D, MLP]), ("w2", [MLP, D]), ("w_ada", [D, 6 * D])]:
        d[nm] = nc.dram_tensor(nm, sh, BF16, kind="ExternalInput").ap()
    for nm, sh in [("b_qkv", [3 * D]), ("b_so", [D]), ("b_cq", [D]), ("b_ck", [D]),
                   ("b_cv", [D]), ("b_co", [D]), ("b1", [MLP]), ("b2", [D]),
                   ("b_ada", [6 * D])]:
        d[nm] = nc.dram_tensor(nm, sh, F32, kind="ExternalInput").ap()
    out = nc.dram_tensor("out_x", [N, D], F32, kind="ExternalOutput").ap()
    srows = nc.dram_tensor("srows", [40, N], F32).ap()
    g_dram = nc.dram_tensor("g_dram", [MT, P, N], BF16).ap()

    tap_shapes = {
        "ada": ([P, 48], F32), "h1": ([P, KD, N], BF16),
        "q": ([P, KD, N], BF16), "k": ([P, KD, N], BF16),
        "v65": ([P, NT, H, 65], BF16), "saO": ([P, KD, N], BF16),
        "x2": ([P, KD, N], F32), "h2": ([P, KD, N], BF16),
        "cq": ([P, KD, N], BF16), "ck": ([P, KD, S], BF16),
        "cv65": ([P, ST, H, 65], BF16), "caO": ([P, KD, N], BF16),
        "x3": ([P, KD, N], F32), "h3": ([P, KD, N], BF16),
        "xT": ([P, KD, N], F32),
    }
    tap_aps = {nm: nc.dram_tensor(f"dbg_{nm}", *tap_shapes[nm], kind="ExternalOutput").ap()
               for nm in taps}

    with tile.TileContext(nc) as tc:
        _emit(nc, tc, d, out, srows, g_dram, tap_aps, upto)
    nc.compile()
    return nc


def _emit(nc, tc, d, out, srows, g_dram, tap_aps={}, upto='full'):
    def tap(nm, t):
        if nm in tap_aps:
            nc.sync.dma_start(tap_aps[nm], t[:])

    gl = contextlib.ExitStack()          # global pools, whole-kernel lifetime
    with gl:
        const = gl.enter_context(tc.tile_pool(name="const", bufs=1))
        resid = gl.enter_context(tc.tile_pool(name="resid", bufs=2))
        wpool = gl.enter_context(tc.tile_pool(name="wpool", bufs=2))
        bigbf = gl.enter_context(tc.tile_pool(name="bigbf", bufs=3))

        # ---------- constants ----------
        ident = const.tile([P, P], F32, tag="ident")
        make_identity(nc, ident)
        ones_bf = const.tile([P, 1], BF16, tag="ones_bf")
        nc.vector.memset(ones_bf[:], 1.0)
        eps_t = const.tile([P, 1], F32, tag="eps")
        nc.vector.memset(eps_t[:], EPS)

        ctxT = const.tile([P, CKD, S], BF16, tag="ctxT")
        ada = const.tile([P, 48], F32, tag="ada")
        splus = const.tile([P, 24], F32, tag="splus")
        xT = resid.tile([P, KD, N], F32, tag="resid")

        def partial_out(ref_tile):
            for k in range(KD):
                nc.sync.dma_start(out[k * P:(k + 1) * P, :], ref_tile[:, k, :])

        # ---------- staging scope ----------
        st = contextlib.ExitStack()
        stg = st.enter_context(tc.tile_pool(name="stg", bufs=2))
        ps_t = st.enter_context(tc.tile_pool(name="ps_t", bufs=2, space="PSUM"))

        def bias_T(name, brow, width):
            stage = stg.tile([width, P], F32, tag="btmp")
            nc.sync.dma_start(stage[:], brow.rearrange("(a p) -> a p", p=P))
            ps = ps_t.tile([P, 512], F32, tag="pst")
            nc.tensor.transpose(ps[:, 0:width], stage[:], ident[0:width, 0:width])
            t = const.tile([P, width], F32, tag=f"bT_{name}")
            nc.vector.tensor_copy(t[:], ps[:, 0:width])
            return t

        b_qkvT = bias_T("qkv", d['b_qkv'], 24)
        b_soT = bias_T("so", d['b_so'], KD)
        b_cqT = bias_T("cq", d['b_cq'], KD)
        b_ckT = bias_T("ck", d['b_ck'], KD)
        b_coT = bias_T("co", d['b_co'], KD)
        b1T = bias_T("b1", d['b1'], MT)
        b_adaT = bias_T("ada", d['b_ada'], 48)
        b2T = bias_T("b2", d['b2'], KD)

        # x -> xT (feature-major, fp32, via PE transpose)
        for i in range(NT):
            xs = stg.tile([P, D], F32, tag="xstage")
            nc.sync.dma_start(xs[:], d['x'][i * P:(i + 1) * P, :])
            for jg in range(2):
                ps = ps_t.tile([P, 512], F32, tag="pst")
                for j4 in range(4):
                    j = jg * 4 + j4
                    nc.tensor.transpose(ps[:, j4 * 128:(j4 + 1) * 128],
                                        xs[:, j * 128:(j + 1) * 128], ident[:])
                nc.vector.tensor_copy(
                    xT[:, jg * 4:(jg + 1) * 4, i * P:(i + 1) * P],
                    ps.rearrange("p (a b) -> p a b", a=4))

        # context -> ctxT (bf16)
        for i in range(ST):
            cs = stg.tile([P, D], F32, tag="xstage")
            nc.sync.dma_start(cs[:, 0:CD], d['context'][i * P:(i + 1) * P, :])
            ps = ps_t.tile([P, 512], F32, tag="pst")
            for j in range(4):
                nc.tensor.transpose(ps[:, j * 128:(j + 1) * 128],
                                    cs[:, j * 128:(j + 1) * 128], ident[:])
            nc.vector.tensor_copy(
                ctxT[:, :, i * P:(i + 1) * P],
                ps.rearrange("p (a b) -> p a b", a=4))

        # c -> silu(c)^T (bf16, feature-major [P, KD])
        cst = stg.tile([KD, P], F32, tag="cstage")
        nc.sync.dma_start(cst[:], d['c'].rearrange("(a p) -> a p", p=P))
        csil = stg.tile([KD, P], F32, tag="cstage")
        nc.scalar.activation(csil[:], cst[:], AF.Silu)
        pcs = ps_t.tile([P, 512], F32, tag="pst")
        nc.tensor.transpose(pcs[:, 0:KD], csil[:], ident[0:KD, 0:KD])
        silu_cT = const.tile([P, KD], BF16, tag="silu_cT")
        nc.vector.tensor_copy(silu_cT[:], pcs[:, 0:KD])

        # ada = silu(c) @ w_ada + b_ada  -> feature-major [P, 48]
        wada = _wcols(d['w_ada'])
        ada_blocks = 0 if upto == 'stage_noada' else 6
        if upto == 'stage_dmaonly':
            # DMA w_ada blocks but skip the matmuls; consume via tiny copy
            for blk in range(6):
                wb = wpool.tile([P, KD, 1024], BF16, tag="wblk")
                nc.sync.dma_start(wb[:], wada[:, :, blk * 1024:(blk + 1) * 1024])
                nc.vector.tensor_copy(ada[:, blk:blk+1].bitcast(BF16)[:, 0:1], wb[:, 0, 0:1])
            ada_blocks = 0
        if ada_blocks == 0:
            nc.vector.memset(ada[:], 0.01)
        for blk in range(ada_blocks):
            wb = wpool.tile([P, KD, 1024], BF16, tag="wblk")
            nc.sync.dma_start(wb[:], wada[:, :, blk * 1024:(blk + 1) * 1024])
            for t8 in range(8):
                t = blk * 8 + t8
                ps = ps_t.tile([P, 512], F32, tag="pst")
                for k in range(KD):
                    nc.tensor.matmul(ps[:, 0:1], wb[:, k, t8 * 128:(t8 + 1) * 128],
                                     silu_cT[:, k:k + 1],
                                     start=(k == 0), stop=(k == KD - 1))
                nc.vector.tensor_copy(ada[:, t:t + 1], ps[:, 0:1])
        nc.vector.tensor_add(ada[:], ada[:], b_adaT[:])
        for g in range(3):
            nc.vector.tensor_scalar_add(splus[:, g * 8:(g + 1) * 8],
                                        ada[:, g * 16 + 8:g * 16 + 16], 1.0)
        tap("ada", ada)
        tap("xT", xT)
        st.close()

        if upto in ('stage', 'stage_noada', 'stage_dmaonly'):
            partial_out(xT)
            return
        # ---------- LN + modulate (self-contained pool scope) ----------
        def ln_mod(x_in, g):
            h_out = bigbf.tile([P, KD, N], BF16, tag="big")
            ls = contextlib.ExitStack()
            with ls:
                lnb = ls.enter_context(tc.tile_pool(name="lnb", bufs=2))
                lrows = ls.enter_context(tc.tile_pool(name="lrows", bufs=3))
                lbc = ls.enter_context(tc.tile_pool(name="lbc", bufs=2))
                ps_ln = ls.enter_context(tc.tile_pool(name="ps_ln", bufs=2, space="PSUM"))
                mps = ps_ln.tile([1, N], F32, tag="lnp")
                sps = ps_ln.tile([1, N], F32, tag="lnp")
                for k in range(KD):
                    xbf = lnb.tile([P, N], BF16, tag="lnxbf")
                    nc.vector.tensor_copy(xbf[:], x_in[:, k])
                    sq = lnb.tile([P, N], BF16, tag="lnsq")
                    nc.vector.tensor_mul(sq[:], xbf[:], xbf[:])
                    for half in range(2):
                        hs = slice(half * 512, (half + 1) * 512)
                        nc.tensor.matmul(mps[:, hs], ones_bf[:], xbf[:, hs],
                                         start=(k == 0), stop=(k == KD - 1))
                        nc.tensor.matmul(sps[:, hs], ones_bf[:], sq[:, hs],
                                         start=(k == 0), stop=(k == KD - 1))
                mu = lrows.tile([1, N], F32, tag="row")
                nc.vector.tensor_scalar_mul(mu[:], mps[:], 1.0 / D)
                var = lrows.tile([1, N], F32, tag="row")
                nc.vector.tensor_scalar_mul(var[:], sps[:], 1.0 / D)
                nc.vector.tensor_mul(mps[:, 0:N], mu[:], mu[:])
                nc.vector.tensor_sub(var[:], var[:], mps[:, 0:N])
                sd = lrows.tile([1, N], F32, tag="row")
                nc.scalar.activation(sd[:], var[:], AF.Sqrt, bias=eps_t[0:1])
                nc.vector.reciprocal(sd[:], sd[:])
                nc.sync.dma_start(srows[2 * g:2 * g + 1, :], mu[:])
                nc.sync.dma_start(srows[2 * g + 1:2 * g + 2, :], sd[:])
                mu_b = lbc.tile([P, N], F32, tag="bcast")
                nc.sync.dma_start(mu_b[:], srows[2 * g:2 * g + 1, :].partition_broadcast(P))
                rstd_b = lbc.tile([P, N], F32, tag="bcast")
                nc.sync.dma_start(rstd_b[:], srows[2 * g + 1:2 * g + 2, :].partition_broadcast(P))
                for k in range(KD):
                    t1 = lnb.tile([P, N], F32, tag="lnt1")
                    nc.vector.tensor_sub(t1[:], x_in[:, k], mu_b[:])
                    nc.vector.tensor_mul(t1[:], t1[:], rstd_b[:])
                    nc.vector.tensor_scalar(h_out[:, k], t1[:],
                                            splus[:, g * 8 + k:g * 8 + k + 1],
                                            ada[:, g * 16 + k:g * 16 + k + 1],
                                            OP.mult, OP.add)
            return h_out

        # ---------- generic transposed projection (512-wide weight blocks) ----
        def proj_T(ps_mm, w_cols, kdin, act_bf, n_free, dout, evict):
            nhalf = max(1, n_free // 512)
            for blk in range(dout // 1024):
                wb = wpool.tile([P, kdin, 1024], BF16, tag="wblk")
                nc.sync.dma_start(wb[:], w_cols[:, :, blk * 1024:(blk + 1) * 1024])
                for t8 in range(8):
                    ps = ps_mm.tile([P, N], F32, tag="pmm")
                    for half in range(nhalf):
                        hs = slice(half * 512, half * 512 + min(512, n_free))
                        for k in range(kdin):
                            nc.tensor.matmul(ps[:, hs],
                                             wb[:, k, t8 * 128:(t8 + 1) * 128],
                                             act_bf[:, k, hs],
                                             start=(k == 0), stop=(k == kdin - 1))
                    evict(blk * 8 + t8, ps)

        # ---------- token-major V projection (fused ones column) ----------
        def proj_V(ps_mm, w_cols, kdin, act_bf, m_tiles, v65, bias_b):
            wb = wpool.tile([P, kdin, 1024], BF16, tag="wblk")
            nc.sync.dma_start(wb[:], w_cols[:])
            for blk in range(2):
                for i in range(m_tiles):
                    ps = ps_mm.tile([P, N], F32, tag="pmm")
                    for k in range(kdin):
                        nc.tensor.matmul(ps[:, 0:512],
                                         act_bf[:, k, i * 128:(i + 1) * 128],
                                         wb[:, k, blk * 512:(blk + 1) * 512],
                                         start=(k == 0), stop=(k == kdin - 1))
                    nc.vector.tensor_add(
                        v65[:, i, blk * 8:(blk + 1) * 8, 0:64],
                        ps[:, 0:512].rearrange("p (h e) -> p h e", h=8),
                        bias_b[:, blk * 512:(blk + 1) * 512]
                        .rearrange("p (h e) -> p h e", h=8))
            nc.vector.memset(v65[:, :, :, 64:65], 1.0)

        # ---------- attention core (self-contained pool scope) ----------
        def attention(q_T, kv_T, v65, m_tiles, o_bf, srow_base):
            at = contextlib.ExitStack()
            with at:
                expp = at.enter_context(tc.tile_pool(name="expp", bufs=2))
                arows = at.enter_context(tc.tile_pool(name="arows", bufs=2))
                rb = at.enter_context(tc.tile_pool(name="rb", bufs=2))
                ps_lg = at.enter_context(tc.tile_pool(name="ps_lg", bufs=2, space="PSUM"))
                ps_pv = at.enter_context(tc.tile_pool(name="ps_pv", bufs=2, space="PSUM"))
                for h in range(H):
                    pr, off = h // 2, (h % 2) * 64
                    pv = ps_pv.tile([65, N], F32, tag="pv")
                    for mt in range(m_tiles):
                        lg = ps_lg.tile([P, N], F32, tag="lg")
                        for half in range(2):
                            hs = slice(half * 512, (half + 1) * 512)
                            nc.tensor.matmul(
                                lg[:, hs],
                                kv_T[off:off + 64, pr, mt * 128:(mt + 1) * 128],
                                q_T[off:off + 64, pr, hs],
                                start=True, stop=True)
                        ex = expp.tile([P, N], BF16, tag="expT")
                        nc.scalar.activation(ex[:], lg[:], AF.Exp, scale=ASCALE)
                        for half in range(2):
                            hs = slice(half * 512, (half + 1) * 512)
                            nc.tensor.matmul(pv[:, hs], v65[:, mt, h, :], ex[:, hs],
                                             start=(mt == 0), stop=(mt == m_tiles - 1))
                    rec = arows.tile([1, N], F32, tag="row")
                    nc.vector.reciprocal(rec[:], pv[64:65, :])
                    r = srow_base + h
                    nc.sync.dma_start(srows[r:r + 1, :], rec[:])
                    rbt = rb.tile([64, N], F32, tag="rbt")
                    nc.sync.dma_start(rbt[:], srows[r:r + 1, :].partition_broadcast(64))
                    nc.vector.tensor_mul(o_bf[off:off + 64, pr, :], pv[0:64, :], rbt[:])

        # ================= self-attention =================
        h1 = ln_mod(xT, 0)

        qT = bigbf.tile([P, KD, N], BF16, tag="big")
        kT = bigbf.tile([P, KD, N], BF16, tag="big")

        sa_es = contextlib.ExitStack()
        vp = sa_es.enter_context(tc.tile_pool(name="vp", bufs=1))
        vb = sa_es.enter_context(tc.tile_pool(name="vb", bufs=1))
        v65 = vp.tile([P, NT, H, 65], BF16, tag="v65")
        vbias_b = vb.tile([P, D], F32, tag="vbias")
        nc.sync.dma_start(vbias_b[:],
                            d['b_qkv'][2 * D:3 * D][None, :].partition_broadcast(P))

        qkv_ps = contextlib.ExitStack()
        ps_mm = qkv_ps.enter_context(tc.tile_pool(name="ps_mm", bufs=2, space="PSUM"))

        def ev_qk(t, ps):
            dst = qT if t < 8 else kT
            nc.scalar.activation(dst[:, t % 8, :], ps[:], AF.Identity,
                                 bias=b_qkvT[:, t:t + 1])
        proj_T(ps_mm, _wcols(d['w_qkv'])[:, :, 0:2 * D], KD, h1, N, 2 * D, ev_qk)
        proj_V(ps_mm, _wcols(d['w_qkv'])[:, :, 2 * D:3 * D], KD, h1, NT, v65, vbias_b)
        tap("h1", h1); tap("q", qT); tap("k", kT); tap("v65", v65)
        qkv_ps.close()

        if upto == 'qkv':
            sa_es.close()
            partial_out(xT)
            return
        saO = bigbf.tile([P, KD, N], BF16, tag="big")
        attention(qT, kT, v65, NT, saO, 6)
        tap("saO", saO)
        sa_es.close()
        if upto == 'sa':
            partial_out(xT)
            return

        x2T = resid.tile([P, KD, N], F32, tag="resid")
        so_ps = contextlib.ExitStack()
        ps_mm = so_ps.enter_context(tc.tile_pool(name="ps_mm", bufs=2, space="PSUM"))

        def ev_so(t, ps):
            nc.vector.tensor_scalar_add(x2T[:, t, :], ps[:], b_soT[:, t:t + 1])
            nc.vector.tensor_add(x2T[:, t, :], x2T[:, t, :], xT[:, t, :])
        proj_T(ps_mm, _wcols(d['w_so']), KD, saO, N, D, ev_so)
        tap("x2", x2T)
        so_ps.close()

        # ================= cross-attention =================
        h2 = ln_mod(x2T, 1)

        cqT = bigbf.tile([P, KD, N], BF16, tag="big")

        ca_es = contextlib.ExitStack()
        kp = ca_es.enter_context(tc.tile_pool(name="kp", bufs=1))
        vp = ca_es.enter_context(tc.tile_pool(name="vp2", bufs=1))
        vb = ca_es.enter_context(tc.tile_pool(name="vb2", bufs=1))
        ckT = kp.tile([P, KD, S], BF16, tag="ckT")
        cv65 = vp.tile([P, ST, H, 65], BF16, tag="cv65")
        cvbias_b = vb.tile([P, D], F32, tag="cvbias")
        nc.sync.dma_start(cvbias_b[:], d['b_cv'][None, :].partition_broadcast(P))

        ca_ps = contextlib.ExitStack()
        ps_mm = ca_ps.enter_context(tc.tile_pool(name="ps_mm", bufs=2, space="PSUM"))

        def ev_cq(t, ps):
            nc.scalar.activation(cqT[:, t, :], ps[:], AF.Identity,
                                 bias=b_cqT[:, t:t + 1])
        proj_T(ps_mm, _wcols(d['w_cq']), KD, h2, N, D, ev_cq)
        tap("h2", h2); tap("cq", cqT)

        def ev_ck(t, ps):
            nc.scalar.activation(ckT[:, t, :], ps[:, 0:S], AF.Identity,
                                 bias=b_ckT[:, t:t + 1])
        proj_T(ps_mm, _wcols(d['w_ck']), CKD, ctxT, S, D, ev_ck)
        proj_V(ps_mm, _wcols(d['w_cv']), CKD, ctxT, ST, cv65, cvbias_b)
        tap("ck", ckT); tap("cv65", cv65)
        ca_ps.close()

        caO = bigbf.tile([P, KD, N], BF16, tag="big")
        attention(cqT, ckT, cv65, ST, caO, 22)
        tap("caO", caO)
        ca_es.close()

        x3T = resid.tile([P, KD, N], F32, tag="resid")
        co_ps = contextlib.ExitStack()
        ps_mm = co_ps.enter_context(tc.tile_pool(name="ps_mm", bufs=2, space="PSUM"))

        def ev_co(t, ps):
            nc.vector.tensor_scalar_add(x3T[:, t, :], ps[:], b_coT[:, t:t + 1])
            nc.vector.tensor_add(x3T[:, t, :], x3T[:, t, :], x2T[:, t, :])
        proj_T(ps_mm, _wcols(d['w_co']), KD, caO, N, D, ev_co)
        tap("x3", x3T)
        co_ps.close()

        if upto == 'ca':
            partial_out(x3T)
            return
        # ================= FFN =================
        h3 = ln_mod(x3T, 2)
        # fold b2 into the residual before the final transpose-accumulate
        for k in range(KD):
            nc.vector.tensor_scalar_add(x3T[:, k, :], x3T[:, k, :], b2T[:, k:k + 1])

        w1_es = contextlib.ExitStack()
        gstage = w1_es.enter_context(tc.tile_pool(name="gstage", bufs=3))
        ps_mm = w1_es.enter_context(tc.tile_pool(name="ps_mm", bufs=2, space="PSUM"))

        def ev_g(t, ps):
            gst = gstage.tile([P, N], BF16, tag="gst")
            nc.scalar.activation(gst[:], ps[:], AF.Gelu, bias=b1T[:, t:t + 1])
            nc.sync.dma_start(g_dram[t], gst[:])
        proj_T(ps_mm, _wcols(d['w1']), KD, h3, N, MLP, ev_g)
        tap("h3", h3)
        w1_es.close()

        if upto == 'w1':
            partial_out(x3T)
            return
        w2_es = contextlib.ExitStack()
        ghp = w2_es.enter_context(tc.tile_pool(name="ghp", bufs=1))
        outst = w2_es.enter_context(tc.tile_pool(name="outst", bufs=2))
        ps_tt = w2_es.enter_context(tc.tile_pool(name="ps_tt", bufs=2, space="PSUM"))
        ps_po = w2_es.enter_context(tc.tile_pool(name="ps_po", bufs=4, space="PSUM"))

        # token-major copy of x3 (+b2), built via PE transposes into bigbf slots
        xtok = [bigbf.tile([P, 4, D], F32, tag="big", name=f"xtok{_i}")
                for _i in range(2)]
        for i in range(NT):
            dst = xtok[i // 4]
            for jg in range(2):
                ps = ps_tt.tile([P, 512], F32, tag="ptt")
                for j4 in range(4):
                    j = jg * 4 + j4
                    nc.tensor.transpose(ps[:, j4 * 128:(j4 + 1) * 128],
                                        x3T[:, j, i * P:(i + 1) * P], ident[:])
                nc.vector.tensor_copy(dst[:, i % 4, jg * 512:(jg + 1) * 512], ps[:])

        w2cols = d['w2'].rearrange("(ko p) f -> p ko f", p=P)
        for nh in range(2):
            gh = ghp.tile([P, MT, 512], BF16, tag="gh")
            nc.sync.dma_start(gh[:], g_dram[:, :, nh * 512:(nh + 1) * 512]
                              .rearrange("m p n -> p m n"))
            for dq in range(4):
                w2q = wpool.tile([P, MT, 256], BF16, tag="wblk")
                nc.sync.dma_start(w2q[:], w2cols[:, :, dq * 256:(dq + 1) * 256])
                for i4 in range(4):
                    i = nh * 4 + i4
                    po = ps_po.tile([P, 256], F32, tag="po")
                    for k in range(MT):
                        nc.tensor.matmul(po[:], gh[:, k, i4 * 128:(i4 + 1) * 128],
                                         w2q[:, k, :],
                                         start=(k == 0), stop=(k == MT - 1))
                    ost = outst.tile([P, 256], F32, tag="ost")
                    nc.vector.tensor_add(
                        ost[:], po[:],
                        xtok[i // 4][:, i % 4, dq * 256:(dq + 1) * 256])
                    nc.sync.dma_start(out[i * P:(i + 1) * P, dq * 256:(dq + 1) * 256],
                                      ost[:])
        w2_es.close()


_NC = None


def _get_nc():
    global _NC
    if _NC is None:
        _NC = build_nc()
    return _NC


def make_in_maps(inputs):
    wnames = ["w_qkv", "w_so", "w_cq", "w_ck", "w_cv", "w_co", "w1", "w2", "w_ada"]
    bnames = ["b_qkv", "b_so", "b_cq", "b_ck", "b_cv", "b_co", "b1", "b2", "b_ada"]
    shared = {}
    for nm in wnames:
        shared[nm] = np.ascontiguousarray(
            np.asarray(inputs[nm]).astype(ml_dtypes.bfloat16))
    for nm in bnames:
        shared[nm] = np.ascontiguousarray(np.asarray(inputs[nm], dtype=np.float32))
    x = np.asarray(inputs['x'], dtype=np.float32)
    c = np.asarray(inputs['c'], dtype=np.float32)
    ctxt = np.asarray(inputs['context'], dtype=np.float32)
    in_maps = []
    for i in range(NCORES):
        m = dict(shared)
        m['x'] = np.ascontiguousarray(x[i])
        m['c'] = np.ascontiguousarray(c[i])
        m['context'] = np.ascontiguousarray(ctxt[i])
        in_maps.append(m)
    return in_maps


def kernel(**inputs):
    nc = _get_nc()
    in_maps = make_in_maps(inputs)
    res = run_bass_kernel_spmd(nc, in_maps, core_ids=list(range(NCORES)))
    return np.stack([res.results[i]["out_x"] for i in range(NCORES)]).astype(np.float32)


if __name__ == "__main__":
    data = np.load("/root/problem/inputs.npz")
    out = kernel(**{k: data[k] for k in data.files})
    gold = np.load("/root/problem/gold64.npy")
    err = np.abs(out - gold)
    print("max abs err:", err.max(), " rel:", err.max() / np.abs(gold).max())


# revision 11
# speedup vs baseline: 4.4616x; 1.4521x over previous
"""Trainium2 Bass kernel: DiT block with cross-attention (nn_DiTBlock_CrossAttn).

Sharding: pure data-parallel over batch. B=8 batch elements -> 8 NeuronCores,
one batch element per core, no collectives. Each core runs the full block:
adaLN -> self-attn -> cross-attn -> FFN (exact GELU).

Layout: activations kept feature-major ("transposed", [feature_part, chunk, token])
so every projection is matmul(lhsT=W[din,dout], rhs=actT[din,n]) with weights in
their natural DRAM layout. Attention uses the S^T orientation with a fused
ones-column in V for the softmax denominator (softmax without max subtraction is
safe: |logits| < ~5 for this problem family). Matmuls run in bf16 (weights are
pre-cast on host), accumulation and residual stream stay fp32.
"""
import contextlib

import numpy as np
import ml_dtypes

import concourse.bass as bass
import concourse.tile as tile
import concourse.mybir as mybir
from concourse import bacc
from concourse.bass_utils import run_bass_kernel_spmd
from concourse.masks import make_identity

P = 128
N = 1024            # tokens
D = 1024            # hidden
KD = D // P         # 8 feature chunks of hidden
NT = N // P         # 8 token tiles
H = 16              # heads
HD = 64             # head dim
S = 256             # context tokens
ST = S // P         # 2
CD = 512            # context dim
CKD = CD // P       # 4
MLP = 4096
MT = MLP // P       # 32
EPS = 1e-6
ASCALE = 0.125      # 1/sqrt(HD)
NCORES = 8

F32 = mybir.dt.float32
BF16 = mybir.dt.bfloat16
AF = mybir.ActivationFunctionType
OP = mybir.AluOpType


def _wcols(w):
    """[din, dout] dram AP -> [p, ko, dout] (feature-chunked lhsT view)."""
    return w.rearrange("(ko p) f -> p ko f", p=P)


def build_nc(taps=(), upto='full'):
    nc = bacc.Bacc("TRN2", target_bir_lowering=False, debug=False)

    d = {}
    d['x'] = nc.dram_tensor("x", [N, D], F32, kind="ExternalInput").ap()
    d['c'] = nc.dram_tensor("c", [D], F32, kind="ExternalInput").ap()
    d['context'] = nc.dram_tensor("context", [S, CD], F32, kind="ExternalInput").ap()
    for nm, sh in [("w_qkv", [D, 3 * D]), ("w_so", [D, D]), ("w_cq", [D, D]),
                   ("w_ck", [CD, D]), ("w_cv", [CD, D]), ("w_co", [D, D]),
                   ("w1", [D, MLP]), ("w2", [MLP, D]), ("w_ada", [D, 6 * D])]:
        d[nm] = nc.dram_tensor(nm, sh, BF16, kind="ExternalInput").ap()
    for nm, sh in [("b_qkv", [3 * D]), ("b_so", [D]), ("b_cq", [D]), ("b_ck", [D]),
                   ("b_cv", [D]), ("b_co", [D]), ("b1", [MLP]), ("b2", [D]),
                   ("b_ada", [6 * D])]:
        d[nm] = nc.dram_tensor(nm, sh, F32, kind="ExternalInput").ap()
    out = nc.dram_tensor("out_x", [N, D], F32, kind="ExternalOutput").ap()
    srows = nc.dram_tensor("srows", [40, N], F32).ap()
    g_dram = nc.dram_tensor("g_dram", [MT, P, N], BF16).ap()

    tap_shapes = {
        "ada": ([P, 48], F32), "h1": ([P, KD, N], BF16),
        "q": ([P, KD, N], BF16), "k": ([P, KD, N], BF16),
        "v65": ([P, NT, H, 65], BF16), "saO": ([P, KD, N], BF16),
        "x2": ([P, KD, N], F32), "h2": ([P, KD, N], BF16),
        "cq": ([P, KD, N], BF16), "ck": ([P, KD, S], BF16),
        "cv65": ([P, ST, H, 65], BF16), "caO": ([P, KD, N], BF16),
        "x3": ([P, KD, N], F32), "h3": ([P, KD, N], BF16),
        "xT": ([P, KD, N], F32),
    }
    tap_aps = {nm: nc.dram_tensor(f"dbg_{nm}", *tap_shapes[nm], kind="ExternalOutput").ap()
               for nm in taps}

    with tile.TileContext(nc) as tc:
        _emit(nc, tc, d, out, srows, g_dram, tap_aps, upto)
    nc.compile()
    return nc


def _emit(nc, tc, d, out, srows, g_dram, tap_aps={}, upto='full'):
    def tap(nm, t):
        if nm in tap_aps:
            nc.sync.dma_start(tap_aps[nm], t[:])

    gl = contextlib.ExitStack()          # global pools, whole-kernel lifetime
    with gl:
        const = gl.enter_context(tc.tile_pool(name="const", bufs=1))
        resid = gl.enter_context(tc.tile_pool(name="resid", bufs=2))
        wpool = gl.enter_context(tc.tile_pool(name="wpool", bufs=2))
        bigbf = gl.enter_context(tc.tile_pool(name="bigbf", bufs=3))

        # ---------- constants ----------
        ident = const.tile([P, P], F32, tag="ident")
        make_identity(nc, ident)
        ones_bf = const.tile([P, 1], BF16, tag="ones_bf")
        nc.vector.memset(ones_bf[:], 1.0)
        eps_t = const.tile([P, 1], F32, tag="eps")
        nc.vector.memset(eps_t[:], EPS)

        ctxT = const.tile([P, CKD, S], BF16, tag="ctxT")
        ada = const.tile([P, 48], F32, tag="ada")
        splus = const.tile([P, 24], F32, tag="splus")
        xT = resid.tile([P, KD, N], F32, tag="resid")

        def partial_out(ref_tile):
            for k in range(KD):
                nc.sync.dma_start(out[k * P:(k + 1) * P, :], ref_tile[:, k, :])

        # ---------- staging scope ----------
        st = contextlib.ExitStack()
        stg = st.enter_context(tc.tile_pool(name="stg", bufs=2))
        ps_t = st.enter_context(tc.tile_pool(name="ps_t", bufs=2, space="PSUM"))

        def bias_T(name, brow, width):
            stage = stg.tile([width, P], F32, tag="btmp")
            nc.sync.dma_start(stage[:], brow.rearrange("(a p) -> a p", p=P))
            ps = ps_t.tile([P, 512], F32, tag="pst")
            nc.tensor.transpose(ps[:, 0:width], stage[:], ident[0:width, 0:width])
            t = const.tile([P, width], F32, tag=f"bT_{name}")
            nc.vector.tensor_copy(t[:], ps[:, 0:width])
            return t

        b_qkvT = bias_T("qkv", d['b_qkv'], 24)
        b_soT = bias_T("so", d['b_so'], KD)
        b_cqT = bias_T("cq", d['b_cq'], KD)
        b_ckT = bias_T("ck", d['b_ck'], KD)
        b_coT = bias_T("co", d['b_co'], KD)
        b1T = bias_T("b1", d['b1'], MT)
        b_adaT = bias_T("ada", d['b_ada'], 48)
        b2T = bias_T("b2", d['b2'], KD)

        # x -> xT (feature-major, fp32, via PE transpose)
        for i in range(NT):
            xs = stg.tile([P, D], F32, tag="xstage")
            nc.sync.dma_start(xs[:], d['x'][i * P:(i + 1) * P, :])
            for jg in range(2):
                ps = ps_t.tile([P, 512], F32, tag="pst")
                for j4 in range(4):
                    j = jg * 4 + j4
                    nc.tensor.transpose(ps[:, j4 * 128:(j4 + 1) * 128],
                                        xs[:, j * 128:(j + 1) * 128], ident[:])
                nc.vector.tensor_copy(
                    xT[:, jg * 4:(jg + 1) * 4, i * P:(i + 1) * P],
                    ps.rearrange("p (a b) -> p a b", a=4))

        # context -> ctxT (bf16)
        for i in range(ST):
            cs = stg.tile([P, D], F32, tag="xstage")
            nc.sync.dma_start(cs[:, 0:CD], d['context'][i * P:(i + 1) * P, :])
            ps = ps_t.tile([P, 512], F32, tag="pst")
            for j in range(4):
                nc.tensor.transpose(ps[:, j * 128:(j + 1) * 128],
                                    cs[:, j * 128:(j + 1) * 128], ident[:])
            nc.vector.tensor_copy(
                ctxT[:, :, i * P:(i + 1) * P],
                ps.rearrange("p (a b) -> p a b", a=4))

        # c -> silu(c)^T (bf16, feature-major [P, KD])
        cst = stg.tile([KD, P], F32, tag="cstage")
        nc.sync.dma_start(cst[:], d['c'].rearrange("(a p) -> a p", p=P))
        csil = stg.tile([KD, P], F32, tag="cstage")
        nc.scalar.activation(csil[:], cst[:], AF.Silu)
        pcs = ps_t.tile([P, 512], F32, tag="pst")
        nc.tensor.transpose(pcs[:, 0:KD], csil[:], ident[0:KD, 0:KD])
        silu_cT = const.tile([P, KD], BF16, tag="silu_cT")
        nc.vector.tensor_copy(silu_cT[:], pcs[:, 0:KD])

        # ada = silu(c) @ w_ada + b_ada  -> feature-major [P, 48]
        wada = _wcols(d['w_ada'])
        ada_blocks = 0 if upto == 'stage_noada' else 6
        if upto == 'stage_dmaonly':
            # DMA w_ada blocks but skip the matmuls; consume via tiny copy
            for blk in range(6):
                wb = wpool.tile([P, KD, 1024], BF16, tag="wblk")
                nc.sync.dma_start(wb[:], wada[:, :, blk * 1024:(blk + 1) * 1024])
                nc.vector.tensor_copy(ada[:, blk:blk+1].bitcast(BF16)[:, 0:1], wb[:, 0, 0:1])
            ada_blocks = 0
        if ada_blocks == 0:
            nc.vector.memset(ada[:], 0.01)
        for blk in range(ada_blocks):
            wb = wpool.tile([P, KD, 1024], BF16, tag="wblk")
            nc.sync.dma_start(wb[:], wada[:, :, blk * 1024:(blk + 1) * 1024])
            for t8 in range(8):
                t = blk * 8 + t8
                ps = ps_t.tile([P, 512], F32, tag="pst")
                for k in range(KD):
                    nc.tensor.matmul(ps[:, 0:1], wb[:, k, t8 * 128:(t8 + 1) * 128],
                                     silu_cT[:, k:k + 1],
                                     start=(k == 0), stop=(k == KD - 1))
                nc.vector.tensor_copy(ada[:, t:t + 1], ps[:, 0:1])
        nc.vector.tensor_add(ada[:], ada[:], b_adaT[:])
        for g in range(3):
            nc.vector.tensor_scalar_add(splus[:, g * 8:(g + 1) * 8],
                                        ada[:, g * 16 + 8:g * 16 + 16], 1.0)
        tap("ada", ada)
        tap("xT", xT)
        st.close()

        if upto in ('stage', 'stage_noada', 'stage_dmaonly'):
            partial_out(xT)
            return
        # ---------- LN + modulate (self-contained pool scope) ----------
        def ln_mod(x_in, g):
            h_out = bigbf.tile([P, KD, N], BF16, tag="big")
            ls = contextlib.ExitStack()
            with ls:
                lnb = ls.enter_context(tc.tile_pool(name="lnb", bufs=2))
                lrows = ls.enter_context(tc.tile_pool(name="lrows", bufs=3))
                lbc = ls.enter_context(tc.tile_pool(name="lbc", bufs=2))
                ps_ln = ls.enter_context(tc.tile_pool(name="ps_ln", bufs=2, space="PSUM"))
                mps = ps_ln.tile([1, N], F32, tag="lnp")
                sps = ps_ln.tile([1, N], F32, tag="lnp")
                for k in range(KD):
                    xbf = lnb.tile([P, N], BF16, tag="lnxbf")
                    nc.vector.tensor_copy(xbf[:], x_in[:, k])
                    sq = lnb.tile([P, N], BF16, tag="lnsq")
                    nc.vector.tensor_mul(sq[:], xbf[:], xbf[:])
                    for half in range(2):
                        hs = slice(half * 512, (half + 1) * 512)
                        nc.tensor.matmul(mps[:, hs], ones_bf[:], xbf[:, hs],
                                         start=(k == 0), stop=(k == KD - 1))
                        nc.tensor.matmul(sps[:, hs], ones_bf[:], sq[:, hs],
                                         start=(k == 0), stop=(k == KD - 1))
                mu = lrows.tile([1, N], F32, tag="row")
                nc.vector.tensor_scalar_mul(mu[:], mps[:], 1.0 / D)
                var = lrows.tile([1, N], F32, tag="row")
                nc.vector.tensor_scalar_mul(var[:], sps[:], 1.0 / D)
                nc.vector.tensor_mul(mps[:, 0:N], mu[:], mu[:])
                nc.vector.tensor_sub(var[:], var[:], mps[:, 0:N])
                sd = lrows.tile([1, N], F32, tag="row")
                nc.scalar.activation(sd[:], var[:], AF.Sqrt, bias=eps_t[0:1])
                nc.vector.reciprocal(sd[:], sd[:])
                nc.sync.dma_start(srows[2 * g:2 * g + 1, :], mu[:])
                nc.sync.dma_start(srows[2 * g + 1:2 * g + 2, :], sd[:])
                mu_b = lbc.tile([P, N], F32, tag="bcast")
                nc.sync.dma_start(mu_b[:], srows[2 * g:2 * g + 1, :].partition_broadcast(P))
                rstd_b = lbc.tile([P, N], F32, tag="bcast")
                nc.sync.dma_start(rstd_b[:], srows[2 * g + 1:2 * g + 2, :].partition_broadcast(P))
                for k in range(KD):
                    t1 = lnb.tile([P, N], F32, tag="lnt1")
                    nc.vector.tensor_sub(t1[:], x_in[:, k], mu_b[:])
                    nc.vector.tensor_mul(t1[:], t1[:], rstd_b[:])
                    nc.vector.tensor_scalar(h_out[:, k], t1[:],
                                            splus[:, g * 8 + k:g * 8 + k + 1],
                                            ada[:, g * 16 + k:g * 16 + k + 1],
                                            OP.mult, OP.add)
            return h_out

        # ---------- generic transposed projection (512-wide weight blocks) ----
        def proj_T(ps_mm, w_cols, kdin, act_bf, n_free, dout, evict):
            nhalf = max(1, n_free // 512)
            for blk in range(dout // 1024):
                wb = wpool.tile([P, kdin, 1024], BF16, tag="wblk")
                nc.sync.dma_start(wb[:], w_cols[:, :, blk * 1024:(blk + 1) * 1024])
                for t8 in range(8):
                    ps = ps_mm.tile([P, N], F32, tag="pmm")
                    for half in range(nhalf):
                        hs = slice(half * 512, half * 512 + min(512, n_free))
                        for k in range(kdin):
                            nc.tensor.matmul(ps[:, hs],
                                             wb[:, k, t8 * 128:(t8 + 1) * 128],
                                             act_bf[:, k, hs],
                                             start=(k == 0), stop=(k == kdin - 1))
                    evict(blk * 8 + t8, ps)

        # ---------- token-major V projection (fused ones column) ----------
        def proj_V(ps_mm, w_cols, kdin, act_bf, m_tiles, v65, bias_b):
            wb = wpool.tile([P, kdin, 1024], BF16, tag="wblk")
            nc.sync.dma_start(wb[:], w_cols[:])
            for blk in range(2):
                for i in range(m_tiles):
                    ps = ps_mm.tile([P, N], F32, tag="pmm")
                    for k in range(kdin):
                        nc.tensor.matmul(ps[:, 0:512],
                                         act_bf[:, k, i * 128:(i + 1) * 128],
                                         wb[:, k, blk * 512:(blk + 1) * 512],
                                         start=(k == 0), stop=(k == kdin - 1))
                    nc.vector.tensor_add(
                        v65[:, i, blk * 8:(blk + 1) * 8, 0:64],
                        ps[:, 0:512].rearrange("p (h e) -> p h e", h=8),
                        bias_b[:, blk * 512:(blk + 1) * 512]
                        .rearrange("p (h e) -> p h e", h=8))
            nc.vector.memset(v65[:, :, :, 64:65], 1.0)

        # ---------- attention core (self-contained pool scope) ----------
        def attention(q_T, kv_T, v65, m_tiles, o_bf, srow_base):
            at = contextlib.ExitStack()
            with at:
                expp = at.enter_context(tc.tile_pool(name="expp", bufs=4))
                arows = at.enter_context(tc.tile_pool(name="arows", bufs=2))
                rb = at.enter_context(tc.tile_pool(name="rb", bufs=2))
                ps_lg = at.enter_context(tc.tile_pool(name="ps_lg", bufs=4, space="PSUM"))
                ps_pv = at.enter_context(tc.tile_pool(name="ps_pv", bufs=2, space="PSUM"))
                for h in range(H):
                    pr, off = h // 2, (h % 2) * 64
                    pv = ps_pv.tile([65, N], F32, tag="pv")
                    # software pipeline over (mt, half) items: logits+exp run two
                    # items ahead of the PV accumulation so the PE never waits on
                    # the ACT exp eviction.
                    items = m_tiles * 2
                    exs = [None] * items
                    def lgexp(i):
                        mt, half = i // 2, i % 2
                        hs = slice(half * 512, (half + 1) * 512)
                        lg = ps_lg.tile([P, 512], F32, tag="lg", name=f"lg{h}_{i}")
                        nc.tensor.matmul(
                            lg[:],
                            kv_T[off:off + 64, pr, mt * 128:(mt + 1) * 128],
                            q_T[off:off + 64, pr, hs],
                            start=True, stop=True)
                        ex = expp.tile([P, 512], BF16, tag="expT", name=f"ex{h}_{i}")
                        nc.scalar.activation(ex[:], lg[:], AF.Exp, scale=ASCALE)
                        exs[i] = ex
                    def pvacc(i):
                        mt, half = i // 2, i % 2
                        hs = slice(half * 512, (half + 1) * 512)
                        nc.tensor.matmul(pv[:, hs], v65[:, mt, h, :], exs[i][:],
                                         start=(mt == 0), stop=(mt == m_tiles - 1))
                    for i in range(items + 2):
                        if i < items:
                            lgexp(i)
                        if i >= 2:
                            pvacc(i - 2)
                    rec = arows.tile([1, N], F32, tag="row")
                    nc.vector.reciprocal(rec[:], pv[64:65, :])
                    r = srow_base + h
                    nc.sync.dma_start(srows[r:r + 1, :], rec[:])
                    rbt = rb.tile([64, N], F32, tag="rbt")
                    nc.sync.dma_start(rbt[:], srows[r:r + 1, :].partition_broadcast(64))
                    nc.vector.tensor_mul(o_bf[off:off + 64, pr, :], pv[0:64, :], rbt[:])

        # ================= self-attention =================
        h1 = ln_mod(xT, 0)

        qT = bigbf.tile([P, KD, N], BF16, tag="big")
        kT = bigbf.tile([P, KD, N], BF16, tag="big")

        sa_es = contextlib.ExitStack()
        vp = sa_es.enter_context(tc.tile_pool(name="vp", bufs=1))
        vb = sa_es.enter_context(tc.tile_pool(name="vb", bufs=1))
        v65 = vp.tile([P, NT, H, 65], BF16, tag="v65")
        vbias_b = vb.tile([P, D], F32, tag="vbias")
        nc.sync.dma_start(vbias_b[:],
                            d['b_qkv'][2 * D:3 * D][None, :].partition_broadcast(P))

        qkv_ps = contextlib.ExitStack()
        ps_mm = qkv_ps.enter_context(tc.tile_pool(name="ps_mm", bufs=2, space="PSUM"))

        def ev_qk(t, ps):
            dst = qT if t < 8 else kT
            nc.scalar.activation(dst[:, t % 8, :], ps[:], AF.Identity,
                                 bias=b_qkvT[:, t:t + 1])
        proj_T(ps_mm, _wcols(d['w_qkv'])[:, :, 0:2 * D], KD, h1, N, 2 * D, ev_qk)
        proj_V(ps_mm, _wcols(d['w_qkv'])[:, :, 2 * D:3 * D], KD, h1, NT, v65, vbias_b)
        tap("h1", h1); tap("q", qT); tap("k", kT); tap("v65", v65)
        qkv_ps.close()

        if upto == 'qkv':
            sa_es.close()
            partial_out(xT)
            return
        saO = bigbf.tile([P, KD, N], BF16, tag="big")
        attention(qT, kT, v65, NT, saO, 6)
        tap("saO", saO)
        sa_es.close()
        if upto == 'sa':
            partial_out(xT)
            return

        x2T = resid.tile([P, KD, N], F32, tag="resid")
        so_ps = contextlib.ExitStack()
        ps_mm = so_ps.enter_context(tc.tile_pool(name="ps_mm", bufs=2, space="PSUM"))

        def ev_so(t, ps):
            nc.vector.tensor_scalar_add(x2T[:, t, :], ps[:], b_soT[:, t:t + 1])
            nc.vector.tensor_add(x2T[:, t, :], x2T[:, t, :], xT[:, t, :])
        proj_T(ps_mm, _wcols(d['w_so']), KD, saO, N, D, ev_so)
        tap("x2", x2T)
        so_ps.close()

        # ================= cross-attention =================
        h2 = ln_mod(x2T, 1)

        cqT = bigbf.tile([P, KD, N], BF16, tag="big")

        ca_es = contextlib.ExitStack()
        kp = ca_es.enter_context(tc.tile_pool(name="kp", bufs=1))
        vp = ca_es.enter_context(tc.tile_pool(name="vp2", bufs=1))
        vb = ca_es.enter_context(tc.tile_pool(name="vb2", bufs=1))
        ckT = kp.tile([P, KD, S], BF16, tag="ckT")
        cv65 = vp.tile([P, ST, H, 65], BF16, tag="cv65")
        cvbias_b = vb.tile([P, D], F32, tag="cvbias")
        nc.sync.dma_start(cvbias_b[:], d['b_cv'][None, :].partition_broadcast(P))

        ca_ps = contextlib.ExitStack()
        ps_mm = ca_ps.enter_context(tc.tile_pool(name="ps_mm", bufs=2, space="PSUM"))

        def ev_cq(t, ps):
            nc.scalar.activation(cqT[:, t, :], ps[:], AF.Identity,
                                 bias=b_cqT[:, t:t + 1])
        proj_T(ps_mm, _wcols(d['w_cq']), KD, h2, N, D, ev_cq)
        tap("h2", h2); tap("cq", cqT)

        def ev_ck(t, ps):
            nc.scalar.activation(ckT[:, t, :], ps[:, 0:S], AF.Identity,
                                 bias=b_ckT[:, t:t + 1])
        proj_T(ps_mm, _wcols(d['w_ck']), CKD, ctxT, S, D, ev_ck)
        proj_V(ps_mm, _wcols(d['w_cv']), CKD, ctxT, ST, cv65, cvbias_b)
        tap("ck", ckT); tap("cv65", cv65)
        ca_ps.close()

        caO = bigbf.tile([P, KD, N], BF16, tag="big")
        attention(cqT, ckT, cv65, ST, caO, 22)
        tap("caO", caO)
        ca_es.close()

        x3T = resid.tile([P, KD, N], F32, tag="resid")
        co_ps = contextlib.ExitStack()
        ps_mm = co_ps.enter_context(tc.tile_pool(name="ps_mm", bufs=2, space="PSUM"))

        def ev_co(t, ps):
            nc.vector.tensor_scalar_add(x3T[:, t, :], ps[:], b_coT[:, t:t + 1])
            nc.vector.tensor_add(x3T[:, t, :], x3T[:, t, :], x2T[:, t, :])
        proj_T(ps_mm, _wcols(d['w_co']), KD, caO, N, D, ev_co)
        tap("x3", x3T)
        co_ps.close()

        if upto == 'ca':
            partial_out(x3T)
            return
        # ================= FFN =================
        h3 = ln_mod(x3T, 2)
        # fold b2 into the residual before the final transpose-accumulate
        for k in range(KD):
            nc.vector.tensor_scalar_add(x3T[:, k, :], x3T[:, k, :], b2T[:, k:k + 1])

        w1_es = contextlib.ExitStack()
        gstage = w1_es.enter_context(tc.tile_pool(name="gstage", bufs=3))
        ps_mm = w1_es.enter_context(tc.tile_pool(name="ps_mm", bufs=2, space="PSUM"))

        def ev_g(t, ps):
            gst = gstage.tile([P, N], BF16, tag="gst")
            nc.scalar.activation(gst[:], ps[:], AF.Gelu, bias=b1T[:, t:t + 1])
            nc.sync.dma_start(g_dram[t], gst[:])
        proj_T(ps_mm, _wcols(d['w1']), KD, h3, N, MLP, ev_g)
        tap("h3", h3)
        w1_es.close()

        if upto == 'w1':
            partial_out(x3T)
            return
        w2_es = contextlib.ExitStack()
        ghp = w2_es.enter_context(tc.tile_pool(name="ghp", bufs=1))
        outst = w2_es.enter_context(tc.tile_pool(name="outst", bufs=2))
        ps_tt = w2_es.enter_context(tc.tile_pool(name="ps_tt", bufs=2, space="PSUM"))
        ps_po = w2_es.enter_context(tc.tile_pool(name="ps_po", bufs=4, space="PSUM"))

        # token-major copy of x3 (+b2), built via PE transposes into bigbf slots
        xtok = [bigbf.tile([P, 4, D], F32, tag="big", name=f"xtok{_i}")
                for _i in range(2)]
        for i in range(NT):
            dst = xtok[i // 4]
            for jg in range(2):
                ps = ps_tt.tile([P, 512], F32, tag="ptt")
                for j4 in range(4):
                    j = jg * 4 + j4
                    nc.tensor.transpose(ps[:, j4 * 128:(j4 + 1) * 128],
                                        x3T[:, j, i * P:(i + 1) * P], ident[:])
                nc.vector.tensor_copy(dst[:, i % 4, jg * 512:(jg + 1) * 512], ps[:])

        w2cols = d['w2'].rearrange("(ko p) f -> p ko f", p=P)
        for nh in range(2):
            gh = ghp.tile([P, MT, 512], BF16, tag="gh")
            for k in range(MT):
                nc.sync.dma_start(gh[:, k, :], g_dram[k, :, nh * 512:(nh + 1) * 512])
            for dq in range(4):
                w2q = wpool.tile([P, MT, 256], BF16, tag="wblk")
                nc.sync.dma_start(w2q[:], w2cols[:, :, dq * 256:(dq + 1) * 256])
                for i4 in range(4):
                    i = nh * 4 + i4
                    po = ps_po.tile([P, 256], F32, tag="po")
                    for k in range(MT):
                        nc.tensor.matmul(po[:], gh[:, k, i4 * 128:(i4 + 1) * 128],
                                         w2q[:, k, :],
                                         start=(k == 0), stop=(k == MT - 1))
                    ost = outst.tile([P, 256], F32, tag="ost")
                    nc.vector.tensor_add(
                        ost[:], po[:],
                        xtok[i // 4][:, i % 4, dq * 256:(dq + 1) * 256])
                    nc.sync.dma_start(out[i * P:(i + 1) * P, dq * 256:(dq + 1) * 256],
                                      ost[:])
        w2_es.close()


_NC = None


def _get_nc():
    global _NC
    if _NC is None:
        _NC = build_nc()
    return _NC


def make_in_maps(inputs):
    wnames = ["w_qkv", "w_so", "w_cq", "w_ck", "w_cv", "w_co", "w1", "w2", "w_ada"]
    bnames = ["b_qkv", "b_so", "b_cq", "b_ck", "b_cv", "b_co", "b1", "b2", "b_ada"]
    shared = {}
    for nm in wnames:
        shared[nm] = np.ascontiguousarray(
            np.asarray(inputs[nm]).astype(ml_dtypes.bfloat16))
    for nm in bnames:
        shared[nm] = np.ascontiguousarray(np.asarray(inputs[nm], dtype=np.float32))
    x = np.asarray(inputs['x'], dtype=np.float32)
    c = np.asarray(inputs['c'], dtype=np.float32)
    ctxt = np.asarray(inputs['context'], dtype=np.float32)
    in_maps = []
    for i in range(NCORES):
        m = dict(shared)
        m['x'] = np.ascontiguousarray(x[i])
        m['c'] = np.ascontiguousarray(c[i])
        m['context'] = np.ascontiguousarray(ctxt[i])
        in_maps.append(m)
    return in_maps


def kernel(**inputs):
    nc = _get_nc()
    in_maps = make_in_maps(inputs)
    res = run_bass_kernel_spmd(nc, in_maps, core_ids=list(range(NCORES)))
    return np.stack([res.results[i]["out_x"] for i in range(NCORES)]).astype(np.float32)


if __name__ == "__main__":
    data = np.load("/root/problem/inputs.npz")
    out = kernel(**{k: data[k] for k in data.files})
    gold = np.load("/root/problem/gold64.npy")
    err = np.abs(out - gold)
    print("max abs err:", err.max(), " rel:", err.max() / np.abs(gold).max())
